# revision 25
# baseline (speedup 1.0000x reference)
"""Trainium2 Bass kernel for GQA attention with ALiBi + sliding window + QK-RMSNorm.

Sharding: SEQUENCE-parallel across 8 cores, zero collectives. Core c owns
512 consecutive query tokens of one batch row (cores 0-3 -> batch 0,
4-7 -> batch 1). Each core redundantly computes K/V projections for its
1536-token sliding-window buffer (own 512 tokens + up to 1024 lookback,
zero-padded at the left edge for the first blocks), so RMSNorm over the
full head axis is core-local and no cross-core communication exists.
Each core computes all 32 q-heads for its tokens and writes a disjoint
[512, 2048] slice of the output; the host just concatenates.

Padding correctness: padded x columns are 0 => K=V=0 there. The relative
(causal+window) mask in the bias table would still admit some padded keys
for early queries, but their softmax contribution is killed by a per-core
"block validity" column (the augmented-V ones column is 0 for padded
blocks), so Z sums only real keys and the numerator gets V=0.

All matmuls are float32r (full PE rate at free-dim>=256). ALiBi+mask are
added to score PSUM via a vector add of a per-head translation-invariant
[128, 1408] table streamed from DRAM (masked = -1e30), then exp on the
scalar engine. Z comes from the augmented-V ones column.
"""
import sys, os
sys.path.insert(0, "/opt/trn_rl_repo")

import numpy as np

B, T, DIM = 2, 2048, 2048
NH, NKV, HD = 32, 8, 64
WINDOW = 1024
EPS = 1e-6
TPC = 512             # query tokens per core
WKV = 1536            # kv window buffer (1024 lookback + 512 own)
NKT = DIM // 128      # 16 contraction tiles
BIAS_W = 1920         # bias cols: u = tq + (t0-s0) + 384
NEG = -1.0e30

_CACHE = {}


def _build_bass():
    from concourse import bass, bacc, mybir
    from concourse.tile import TileContext

    dt = mybir.dt.float32
    dtr = mybir.dt.float32r
    AF = mybir.ActivationFunctionType

    nc = bacc.Bacc("TRN2", target_bir_lowering=False, debug=False,
                   num_devices=8)

    xw_d = nc.dram_tensor("xw", [DIM, WINDOW], dtr, kind="ExternalInput")
    xq_d = nc.dram_tensor("xq", [DIM, TPC], dtr, kind="ExternalInput")
    wkv_d = nc.dram_tensor("wkvT", [DIM, 1024], dtr, kind="ExternalInput")
    wq_d = nc.dram_tensor("wqT", [DIM, DIM], dtr, kind="ExternalInput")
    wo_d = nc.dram_tensor("woT", [DIM, DIM], dtr, kind="ExternalInput")
    qnw_d = nc.dram_tensor("qnw", [1, DIM], dtr, kind="ExternalInput")
    knw_d = nc.dram_tensor("knw", [1, 512], dtr, kind="ExternalInput")
    bias_d = nc.dram_tensor("biasT", [NH, 128, BIAS_W], dtr,
                            kind="ExternalInput")
    onesc_d = nc.dram_tensor("ones_blk", [WKV, 1], dtr, kind="ExternalInput")
    ones2_d = nc.dram_tensor("ones2", [2, 128], dtr, kind="ExternalInput")
    onescol_d = nc.dram_tensor("ones_col", [128, 1], dtr,
                               kind="ExternalInput")
    out_d = nc.dram_tensor("out", [TPC, DIM], dt, kind="ExternalOutput")

    NBLK = WKV // 128  # 12 kv blocks

    with TileContext(nc) as tc:
        P = tc.alloc_tile_pool

        cp = P(name="consts", bufs=1)
        ones2 = cp.tile([2, 128], dtr, tag="on2", name="on2")
        nc.sync.dma_start(ones2[:], ones2_d[:])
        onescol = cp.tile([128, 1], dtr, tag="onc", name="onc")
        nc.sync.dma_start(onescol[:], onescol_d[:])
        # block-validity column packed [128, 12]
        ones_sb = cp.tile([128, NBLK], dtr, tag="ob", name="ob")
        nc.sync.dma_start(
            ones_sb[:], onesc_d[:].rearrange("(b p) o -> p (b o)", p=128))

        # persistent activations for phases A-C
        app = P(name="acts", bufs=1)
        kts = [app.tile([128, WKV], dtr, tag=f"k{m}", name=f"k{m}")
               for m in range(4)]
        vaug = [[app.tile([128, HD + 1], dtr, tag=f"v{h}_{sb}",
                          name=f"v{h}_{sb}")
                 for sb in range(NBLK)] for h in range(NKV)]
        for h in range(NKV):
            for sb in range(NBLK):
                nc.vector.tensor_copy(vaug[h][sb][:, HD:HD + 1],
                                      ones_sb[:, sb:sb + 1])
        rsk_raw = app.tile([1, WKV], dt, tag="rskr", name="rskr")
        rsq_raw = app.tile([1, TPC], dt, tag="rsqr", name="rsqr")

        xp = P(name="xinp", bufs=1)  # x tiles, lifetime A..B

        # ============ phase A: K/V projection + sumsq ============
        # K out-stationary over contraction k (4 PSUM banks); V computed
        # directly in [token, vdim] layout (no PE transposes).
        wp = P(name="wkvp", bufs=1)
        kp = P(name="knwp", bufs=1)
        wk = P(name="wkA", bufs=2)
        nk = P(name="nwtK", bufs=1)
        pjk = P(name="psK", bufs=1, space="PSUM")
        pjv = P(name="psV", bufs=2, space="PSUM")
        pss = P(name="psS", bufs=1, space="PSUM")

        knw_sb = kp.tile([1, 512], dtr, tag="knw", name="knw")
        nc.sync.dma_start(knw_sb[:], knw_d[:])
        wkv = [wp.tile([128, 1024], dtr, tag=f"w{k}", name=f"w{k}")
               for k in range(NKT)]
        xqts = None
        for tcn in range(3):
            csl = slice(tcn * 512, (tcn + 1) * 512)
            xts = []
            for k in range(NKT):
                if tcn == 0:
                    # interleave weight/x loads so matmul k can start
                    # as soon as its own operands land
                    nc.sync.dma_start(wkv[k][:],
                                      wkv_d[k * 128:(k + 1) * 128, :])
                t = xp.tile([128, 512], dtr, tag=f"x{k}", name=f"x{k}")
                if tcn < 2:
                    nc.sync.dma_start(t[:], xw_d[k * 128:(k + 1) * 128, csl])
                else:
                    nc.sync.dma_start(t[:], xq_d[k * 128:(k + 1) * 128, :])
                xts.append(t)
            if tcn == 2:
                xqts = xts
            kpsums = [pjk.tile([128, 512], dt, tag=f"pk{mt}",
                               name=f"pk{mt}")
                      for mt in range(4)]
            for k in range(NKT):
                for mt in range(4):
                    nc.tensor.matmul(
                        kpsums[mt][:], wkv[k][:, mt * 128:(mt + 1) * 128],
                        xts[k][:], start=(k == 0), stop=(k == NKT - 1))
            sskp = pss.tile([1, 512], dt, tag="ssk", name="sskp")
            for mt in range(4):
                nc.gpsimd.tensor_copy(kts[mt][:, csl], kpsums[mt][:])
                sq = wk.tile([128, 512], dtr, tag="sq", name="sqA")
                nc.vector.tensor_mul(sq[:], kts[mt][:, csl], kts[mt][:, csl])
                nc.tensor.matmul(sskp[:], onescol[:], sq[:],
                                 start=(mt == 0), stop=(mt == 3))
            nc.vector.tensor_copy(rsk_raw[0:1, csl], sskp[:])
            for tt in range(4):
                vps = pjv.tile([128, 512], dt, tag="pv", name="vps")
                for k in range(NKT):
                    nc.tensor.matmul(
                        vps[:], xts[k][:, tt * 128:(tt + 1) * 128],
                        wkv[k][:, 512:1024],
                        start=(k == 0), stop=(k == NKT - 1))
                blk = tcn * 4 + tt
                for h in range(NKV):
                    nc.vector.tensor_copy(vaug[h][blk][:, 0:HD],
                                          vps[:, h * 64:(h + 1) * 64])

        # ---- k rsqrt (Newton-refined) + normalize ----
        for p in (pss, pjv, pjk):
            p.release()
        pnk = P(name="psN", bufs=2, space="PSUM")
        vak = nk.tile([1, WKV], dt, tag="vak", name="vak")
        nc.scalar.activation(vak[:], rsk_raw[:], AF.Copy,
                             bias=float(EPS), scale=1.0 / 512.0)
        s1k = nk.tile([1, WKV], dt, tag="s1k", name="s1k")
        nc.scalar.activation(s1k[:], vak[:], AF.Sqrt)
        y0k = nk.tile([1, WKV], dt, tag="y0k", name="y0k")
        nc.vector.reciprocal(y0k[:], s1k[:])
        nc.vector.tensor_mul(s1k[:], y0k[:], y0k[:])
        nc.vector.tensor_mul(s1k[:], s1k[:], vak[:])
        nc.scalar.activation(s1k[:], s1k[:], AF.Copy, bias=1.5, scale=-0.5)
        rskf = nk.tile([1, WKV], dtr, tag="rskf", name="rskf")
        nc.vector.tensor_mul(rskf[:], y0k[:], s1k[:])
        for mt in range(4):
            for tcn in range(3):
                csl = slice(tcn * 512, (tcn + 1) * 512)
                scpk = pnk.tile([128, 512], dt, tag="nk", name="scpk")
                nc.tensor.matmul(scpk[:],
                                 knw_sb[0:1, mt * 128:(mt + 1) * 128],
                                 rskf[0:1, csl], start=True, stop=True)
                nc.vector.tensor_mul(kts[mt][:, csl], kts[mt][:, csl],
                                     scpk[:])
        for p in (pnk, nk, wk, kp, wp):
            p.release()

        # ============ phase B: Q projection + norm ============
        otp = P(name="oTp", bufs=1, side="right")     # lives B..D
        qtp = P(name="qtsp", bufs=1, side="right")    # lives B..C
        qts = [qtp.tile([128, TPC], dtr, tag=f"q{m}", name=f"q{m}")
               for m in range(NKT)]
        wqp = P(name="wqp", bufs=1)
        qp = P(name="qnwp", bufs=1)
        wkb = P(name="wkB", bufs=2)
        nq = P(name="nwtQ", bufs=1)
        pjb = P(name="psB", bufs=1, space="PSUM")
        psq = P(name="psSq", bufs=1, space="PSUM")
        pnq = P(name="psNq", bufs=2, space="PSUM")

        qnw_sb = qp.tile([1, DIM], dtr, tag="qnw", name="qnw")
        nc.sync.dma_start(qnw_sb[:], qnw_d[:])
        ssqp = psq.tile([1, TPC], dt, tag="ssq", name="ssqp")
        for mg in range(8):
            wqt = []
            for k in range(NKT):
                t = wqp.tile([128, 256], dtr, tag=f"wq{k}", name=f"wq{k}")
                nc.sync.dma_start(
                    t[:], wq_d[k * 128:(k + 1) * 128,
                               mg * 256:(mg + 1) * 256])
                wqt.append(t)
            qpsums = [pjb.tile([128, TPC], dt, tag=f"pq{ml}",
                               name=f"pq{ml}")
                      for ml in range(2)]
            for k in range(NKT):
                for ml in range(2):
                    nc.tensor.matmul(
                        qpsums[ml][:], wqt[k][:, ml * 128:(ml + 1) * 128],
                        xqts[k][:], start=(k == 0), stop=(k == NKT - 1))
            for ml in range(2):
                m = mg * 2 + ml
                nc.gpsimd.tensor_copy(qts[m][:], qpsums[ml][:])
                sq = wkb.tile([128, TPC], dtr, tag="sq", name="sqB")
                nc.vector.tensor_mul(sq[:], qts[m][:], qts[m][:])
                nc.tensor.matmul(ssqp[:], onescol[:], sq[:],
                                 start=(m == 0), stop=(m == 15))
        nc.vector.tensor_copy(rsq_raw[:], ssqp[:])
        # q rsqrt: folds the 1/8 score scale (sc=1/32, bi=64eps)
        vaq = nq.tile([1, TPC], dt, tag="vaq", name="vaq")
        nc.scalar.activation(vaq[:], rsq_raw[:], AF.Copy,
                             bias=float(64.0 * EPS), scale=1.0 / 32.0)
        s1q = nq.tile([1, TPC], dt, tag="s1q", name="s1q")
        nc.scalar.activation(s1q[:], vaq[:], AF.Sqrt)
        y0q = rsq_raw
        nc.vector.reciprocal(y0q[:], s1q[:])
        nc.vector.tensor_mul(s1q[:], y0q[:], y0q[:])
        nc.vector.tensor_mul(s1q[:], s1q[:], vaq[:])
        nc.scalar.activation(s1q[:], s1q[:], AF.Copy, bias=1.5, scale=-0.5)
        rsqf = nq.tile([1, TPC], dtr, tag="rsqf", name="rsqf")
        nc.vector.tensor_mul(rsqf[:], y0q[:], s1q[:])
        for m in range(NKT):
            scpq = pnq.tile([128, TPC], dt, tag="nq", name="scpq")
            nc.tensor.matmul(scpq[:], qnw_sb[0:1, m * 128:(m + 1) * 128],
                             rsqf[0:1, :], start=True, stop=True)
            nc.vector.tensor_mul(qts[m][:], qts[m][:], scpq[:])
        for p in (pnq, psq, pjb, nq, wkb, qp, wqp):
            p.release()
        xp.release()

        # ============ phase C: attention (full-width q blocks) ============
        oT = [otp.tile([128, TPC], dtr, tag=f"o{m}", name=f"o{m}")
              for m in range(NKT)]
        bp = P(name="biasp", bufs=2)
        ep = P(name="expp", bufs=6)
        wkc = P(name="wkC", bufs=3)
        psc = P(name="psSc", bufs=3, space="PSUM")
        po = P(name="psO", bufs=2, space="PSUM")
        pz = P(name="psZ", bufs=2, space="PSUM")
        for h in range(NH):
            g, r = divmod(h, 8)
            ktile = kts[g]
            kvh = h // 4
            qrow = (r // 4) * 64
            krow = qrow
            qtile = qts[4 * g + (r % 4)]
            bt = bp.tile([128, BIAS_W], dtr, tag="b", name="bt")
            nc.sync.dma_start(bt[:], bias_d[h])
            opsum = po.tile([HD + 1, TPC], dt, tag="o", name="opsum")
            for j in range(NBLK):
                s0 = j * 128
                scp = psc.tile([128, TPC], dt, tag="sc", name="scp")
                nc.tensor.matmul(
                    scp[:], ktile[krow:krow + 64, s0:s0 + 128],
                    qtile[qrow:qrow + 64, :], start=True, stop=True)
                u0 = 1408 - 128 * j
                et = ep.tile([128, TPC], dtr, tag="e", name="et")
                eng = nc.vector if j % 2 else nc.gpsimd
                eng.tensor_add(et[:], scp[:], bt[:, u0:u0 + TPC])
                nc.scalar.activation(et[:], et[:], AF.Exp)
                nc.tensor.matmul(opsum[:], vaug[kvh][j][:], et[:],
                                 start=(j == 0), stop=(j == NBLK - 1))
            zf = wkc.tile([1, TPC], dt, tag="zf", name="zf")
            nc.vector.reciprocal(zf[:], opsum[HD:HD + 1, :])
            zr = wkc.tile([1, TPC], dtr, tag="zr", name="zr")
            nc.vector.tensor_copy(zr[:], zf[:])
            zbc = pz.tile([64, TPC], dt, tag="zb", name="zbc")
            nc.tensor.matmul(zbc[:], ones2[0:1, 0:64], zr[:],
                             start=True, stop=True)
            osl = oT[4 * g + (r % 4)][qrow:qrow + 64, :]
            nc.vector.tensor_copy(osl, opsum[0:HD, :])
            nc.vector.tensor_mul(osl, osl, zbc[:])
        for p in (pz, po, psc, wkc, ep, bp):
            p.release()
        qtp.release()
        app.release()

        # ============ phase D: output projection ============
        wop = P(name="wop", bufs=2)
        osp = P(name="ostp", bufs=3)
        pjd = P(name="psD", bufs=1, space="PSUM")
        for fc in range(4):
            fsl = slice(fc * 512, (fc + 1) * 512)
            wot = []
            for k in range(NKT):
                t = wop.tile([128, 512], dtr, tag=f"wo{k}", name=f"wo{k}")
                nc.sync.dma_start(t[:], wo_d[k * 128:(k + 1) * 128, fsl])
                wot.append(t)
            wpsums = [pjd.tile([128, 512], dt, tag=f"pd{tt}",
                               name=f"pd{tt}")
                      for tt in range(4)]
            for k in range(NKT):
                for tt in range(4):
                    nc.tensor.matmul(
                        wpsums[tt][:], oT[k][:, tt * 128:(tt + 1) * 128],
                        wot[k][:], start=(k == 0), stop=(k == NKT - 1))
            for tt in range(4):
                ost = osp.tile([128, 512], dt, tag="os", name="ost")
                nc.gpsimd.tensor_copy(ost[:], wpsums[tt][:])
                nc.sync.dma_start(out_d[tt * 128:(tt + 1) * 128, fsl],
                                  ost[:])
        for p in (pjd, osp, wop):
            p.release()
        otp.release()
        cp.release()
    nc.finalize()
    return nc


def _host_inputs(x, wq, wk, wv, wo, q_norm_w, k_norm_w):
    f32 = np.float32
    x = np.asarray(x, f32)
    wq = np.asarray(wq, f32)
    wk = np.asarray(wk, f32)
    wv = np.asarray(wv, f32)
    wo = np.asarray(wo, f32)
    r = 2.0 ** (-8.0 / NH)
    slopes = np.asarray([r ** i for i in range(NH)], f32)
    ones2 = np.ones((2, 128), f32)
    onescol = np.ones((128, 1), f32)
    wkvT = np.ascontiguousarray(np.concatenate([wk, wv], 0).T)
    # device q/o head layout: tile m=4g+i holds head 8g+i in rows 0:64 and
    # head 8g+i+4 in rows 64:128, so each q-head's partition half matches
    # its kv-head's half in the packed K tiles
    perm = np.empty(DIM, np.int64)
    ar = np.arange(64)
    for m in range(NKT):
        g, i = divmod(m, 4)
        perm[m * 128:m * 128 + 64] = (8 * g + i) * 64 + ar
        perm[m * 128 + 64:(m + 1) * 128] = (8 * g + i + 4) * 64 + ar
    wqT = np.ascontiguousarray(wq.T[:, perm])
    woT = np.ascontiguousarray(wo.T[perm, :])
    qnw = np.asarray(q_norm_w, f32).reshape(1, DIM)[:, perm]
    knw = np.asarray(k_norm_w, f32).reshape(1, 512)
    # per-head translation-invariant bias table (shared by all cores)
    ds = np.arange(128, dtype=np.int64)[:, None]
    ui = np.arange(BIAS_W, dtype=np.int64)[None, :] - 384
    dist = ui - ds  # local (t - s)
    allowed = (dist >= 0) & (dist <= WINDOW)
    bias = np.empty((NH, 128, BIAS_W), f32)
    for h in range(NH):
        bias[h] = np.where(allowed, (-slopes[h] * dist).astype(f32), f32(NEG))
    in_maps = []
    for c in range(8):
        b, blk = divmod(c, 4)
        t0 = blk * TPC
        pad = max(0, WINDOW - t0)
        xwin = np.zeros((WINDOW, DIM), f32)
        if pad < WINDOW:
            xwin[pad:] = x[b, t0 - (WINDOW - pad):t0]
        onesb = np.ones((WKV, 1), f32)
        onesb[:pad] = 0.0
        in_maps.append({
            "xw": np.ascontiguousarray(xwin.T),
            "xq": np.ascontiguousarray(x[b, t0:t0 + TPC].T),
            "wkvT": wkvT,
            "wqT": wqT,
            "woT": woT,
            "qnw": qnw,
            "knw": knw,
            "biasT": bias,
            "ones_blk": onesb,
            "ones2": ones2,
            "ones_col": onescol,
        })
    return in_maps


def kernel(x, wq, wk, wv, wo, q_norm_w, k_norm_w):
    from concourse.bass_utils import run_bass_kernel_spmd
    if "nc" not in _CACHE:
        _CACHE["nc"] = _build_bass()
    nc = _CACHE["nc"]
    in_maps = _host_inputs(x, wq, wk, wv, wo, q_norm_w, k_norm_w)
    res = run_bass_kernel_spmd(nc, in_maps, core_ids=list(range(8)))
    out = np.empty((B, T, DIM), np.float32)
    for c in range(8):
        b, blk = divmod(c, 4)
        out[b, blk * TPC:(blk + 1) * TPC] = res.results[c]["out"]
    return out
